# revision 17
# baseline (speedup 1.0000x reference)
"""Trainium2 Bass kernel for per-combination linear encoder (embedding lookup).

Computes z = y * w[idx] + b[idx] where idx = t*1024 + x @ [512,256,...,1]
for x in {0,1}^[N,10], t in {0,1}^[N,1], over a 2048-entry (w,b) table.

Sharding: data-parallel over the batch axis across 8 NeuronCores.

Hybrid kernel: the stock GPSIMD ap_gather costs ~28 ns/idx (latency-bound
in the Q7 request loop), walling a gather-only kernel at ~875 us/core.
Each core therefore splits rows across two concurrent pipelines, balanced
so both finish together (the transposed path measures ~12.3 ns/row, so it
takes ~22% of the rows):

G-path (GPSIMD gather, row-major [128, B] tiles), R_G rows:
  DMA (t,x) fp16 -> DVE idx -> ap_gather fp16 (w,b) pairs -> PE un-wrap
  (32 fp16 diagonal-mask matmuls) -> DVE FMA -> DMA out.

T-path (PE/ACT/DVE, transposed layout: rows on the free dim), R_T rows:
  idx = hi*64 + lo (hi: t,x0..x3 at partitions 0:32; lo: x4..x9 at
  64:128).  Host uploads bits transposed [13, R_T] (11 bits, ones, y).
  PE1a: q[j,r] = match-count trick (integer <= 0, ==0 iff combo j matches)
  PE1b: yrep[64:128, r] = y[r]
  ACT:  oh = Relu(q + 1)            exact 0/1 one-hot
  DVE:  ohly = oh_lo * yrep
  PE2:  A[h,r] = Ww2^T ohly + Wb2^T oh_lo
  DVE:  m = oh_hi * A
  PE3:  z = ones^T m;  ACT copies PSUM->SBUF, DMA out.
"""

import numpy as np

import concourse.bacc as bacc
import concourse.mybir as mybir
from concourse.tile import TileContext
from concourse.bass_utils import run_bass_kernel_spmd

M = 8            # NeuronCores
P = 128          # SBUF partitions
D = 11           # bits per row: (t, x0..x9)
C = 2048         # table entries
F16 = mybir.dt.float16
F32 = mybir.dt.float32
I16 = mybir.dt.int16

# G-path schedule (rows per partition per tile)
B_SCHED = (48, 330, 330, 330, 330, 106, 48)
RPP = sum(B_SCHED)          # 1522
R_G = P * RPP               # 194_816 rows per core

# T-path: chunks of F rows (PSUM tiles are one 2 KB bank at F=512)
F = 512
NCHUNK = 108
R_T = F * NCHUNK            # 55_296 rows per core

R_CORE = 250_000            # real rows per core (N / 8)
assert R_G + R_T >= R_CORE

_CACHE = {}


def _build_program():
    nc = bacc.Bacc("TRN2", target_bir_lowering=False, debug=False, num_devices=M)

    xt = nc.dram_tensor("xt", [R_G, D], F16, kind="ExternalInput")
    y = nc.dram_tensor("y", [R_G], F16, kind="ExternalInput")
    wb = nc.dram_tensor("wb", [P, 2 * C], F16, kind="ExternalInput")
    pw = nc.dram_tensor("pw", [P, D], F16, kind="ExternalInput")
    mk = nc.dram_tensor("mk", [P, 16 * P], F16, kind="ExternalInput")
    bt = nc.dram_tensor("bt", [13, R_T], F16, kind="ExternalInput")
    l1 = nc.dram_tensor("l1", [13, 128], F16, kind="ExternalInput")
    ly = nc.dram_tensor("ly", [13, 128], F16, kind="ExternalInput")
    t2w = nc.dram_tensor("t2w", [64, 32], F16, kind="ExternalInput")
    t2b = nc.dram_tensor("t2b", [64, 32], F16, kind="ExternalInput")
    on32 = nc.dram_tensor("on32", [32, 1], F16, kind="ExternalInput")
    z = nc.dram_tensor("z", [R_G], F16, kind="ExternalOutput")
    zt2 = nc.dram_tensor("zt2", [R_T], F16, kind="ExternalOutput")

    x3 = xt.ap().rearrange("(pp r) d -> pp (r d)", pp=P)   # [P, RPP*D]
    y2 = y.ap().rearrange("(pp r) -> pp r", pp=P)          # [P, RPP]
    z2 = z.ap().rearrange("(pp r) -> pp r", pp=P)
    z2t = zt2.ap().rearrange("(c f) -> c f", f=F)          # [NCHUNK, F]

    with TileContext(nc) as tc:
        with (
            tc.tile_pool(name="const", bufs=1) as cpool,
            tc.tile_pool(name="sb", bufs=3) as pool,
            tc.tile_pool(name="gat", bufs=3) as gpool,
            tc.tile_pool(name="tsb", bufs=2) as tpool,
            tc.tile_pool(name="ps2", bufs=2, space="PSUM") as ppool,
            tc.tile_pool(name="ps1", bufs=1, space="PSUM") as qpool,
        ):
            wb_t = cpool.tile([P, 2 * C], F16)
            nc.sync.dma_start(out=wb_t[:], in_=wb[:, :])
            pw_t = cpool.tile([P, D], F16)
            nc.sync.dma_start(out=pw_t[:], in_=pw[:, :])
            mk_t = cpool.tile([P, 16 * P], F16)
            nc.sync.dma_start(out=mk_t[:], in_=mk[:, :])
            l1_t = cpool.tile([13, 128], F16)
            nc.sync.dma_start(out=l1_t[:], in_=l1[:, :])
            ly_t = cpool.tile([13, 128], F16)
            nc.sync.dma_start(out=ly_t[:], in_=ly[:, :])
            t2w_t = cpool.tile([P, 32], F16)
            nc.sync.dma_start(out=t2w_t[64:128, :], in_=t2w[:, :])
            t2b_t = cpool.tile([P, 32], F16)
            nc.sync.dma_start(out=t2b_t[64:128, :], in_=t2b[:, :])
            on32_t = cpool.tile([32, 1], F16)
            nc.sync.dma_start(out=on32_t[:], in_=on32[:, :])

        # ---- T-path chunk emitters --------------------------------------
            def t_pair(c0):
                btt = tpool.tile([13, 2 * F], F16, tag="bt")
                nc.sync.dma_start(out=btt[:], in_=bt[:, c0 * F:(c0 + 2) * F])
                for k in (0, 1):
                    t_chunk(c0 + k, btt[:, k * F:(k + 1) * F])

            def t_chunk(c, btc):
                q = qpool.tile([128, F], F32, tag="q")
                nc.tensor.matmul(out=q[:], lhsT=l1_t[:], rhs=btc,
                                 start=True, stop=True)
                yrep = qpool.tile([128, F], F32, tag="yrep")
                nc.tensor.matmul(out=yrep[:], lhsT=ly_t[:], rhs=btc,
                                 start=True, stop=True)

                oh = tpool.tile([128, F], F16, tag="oh")
                nc.scalar.activation(
                    out=oh[:], in_=q[:],
                    func=mybir.ActivationFunctionType.Relu, bias=1.0, scale=1.0,
                )

                ohly = tpool.tile([128, F], F16, tag="ohly")
                nc.vector.tensor_tensor(
                    out=ohly[64:128, :], in0=oh[64:128, :], in1=yrep[64:128, :],
                    op=mybir.AluOpType.mult,
                )

                acc = qpool.tile([32, F], F32, tag="acc")
                nc.tensor.matmul(out=acc[:], lhsT=t2w_t[64:128, :],
                                 rhs=ohly[64:128, :], start=True, stop=False)
                nc.tensor.matmul(out=acc[:], lhsT=t2b_t[64:128, :],
                                 rhs=oh[64:128, :], start=False, stop=True)

                m = tpool.tile([32, F], F16, tag="m")
                nc.vector.tensor_tensor(
                    out=m[:], in0=oh[0:32, :], in1=acc[:],
                    op=mybir.AluOpType.mult,
                )

                zp = qpool.tile([1, F], F32, tag="zp")
                nc.tensor.matmul(out=zp[:], lhsT=on32_t[:], rhs=m[:],
                                 start=True, stop=True)
                ztile = tpool.tile([1, F], F16, tag="zt")
                nc.scalar.copy(out=ztile[:], in_=zp[:])
                nc.sync.dma_start(out=z2t[c, :], in_=ztile[:])

        # ---- G-path tiles, with T-chunk pairs interleaved ---------------
            nt = len(B_SCHED)
            npair = NCHUNK // 2
            per = -(-npair // max(1, nt - 1))
            off = 0
            cnext = 0
            for ti, B in enumerate(B_SCHED):
                xtt = pool.tile([P, B * D], F16, tag="x")
                nc.sync.dma_start(out=xtt[:], in_=x3[:, off * D:(off + B) * D])
                yt = pool.tile([P, B], F16, tag="y")
                nc.sync.dma_start(out=yt[:], in_=y2[:, off:off + B])

                xv = xtt[:].rearrange("p (b d) -> p b d", d=D)
                nc.vector.tensor_tensor(
                    out=xv, in0=xv,
                    in1=pw_t[:].unsqueeze(1).broadcast_to([P, B, D]),
                    op=mybir.AluOpType.mult,
                )
                idxf = pool.tile([P, B], F32, tag="idxf")
                nc.vector.tensor_reduce(
                    out=idxf[:], in_=xv, axis=mybir.AxisListType.X,
                    op=mybir.AluOpType.add,
                )
                idx16 = pool.tile([P, B], I16, tag="idx16")
                nc.vector.tensor_copy(out=idx16[:], in_=idxf[:])

                og = gpool.tile([P, 16 * B * 2], F16, tag="og")
                nc.gpsimd.ap_gather(
                    out_ap=og[:].rearrange("p (j e) -> p j e", e=2),
                    in_ap=wb_t[:].rearrange("p (c e) -> p c e", e=2),
                    idxs_ap=idx16[:],
                    channels=P, num_elems=C, d=2, num_idxs=16 * B,
                )

                og3 = og[:].rearrange("p (c s) -> p c s", s=32)
                psw = ppool.tile([P, B], F32, tag="psw")
                psb = ppool.tile([P, B], F32, tag="psb")
                for qq in range(16):
                    nc.tensor.matmul(
                        out=psw[:], lhsT=mk_t[:, qq * P:(qq + 1) * P],
                        rhs=og3[:, :, 2 * qq], start=(qq == 0), stop=(qq == 15),
                    )
                for qq in range(16):
                    nc.tensor.matmul(
                        out=psb[:], lhsT=mk_t[:, qq * P:(qq + 1) * P],
                        rhs=og3[:, :, 2 * qq + 1], start=(qq == 0), stop=(qq == 15),
                    )

                zg = pool.tile([P, B], F16, tag="z")
                nc.vector.tensor_tensor(
                    out=zg[:], in0=yt[:], in1=psw[:], op=mybir.AluOpType.mult
                )
                nc.vector.tensor_tensor(
                    out=zg[:], in0=zg[:], in1=psb[:], op=mybir.AluOpType.add
                )
                nc.sync.dma_start(out=z2[:, off:off + B], in_=zg[:])
                off += B

                if ti >= 1:
                    stop_c = min(npair, cnext + per) if ti < nt - 1 else npair
                    while cnext < stop_c:
                        t_pair(2 * cnext)
                        cnext += 1

    nc.compile()
    return nc


def _get_program():
    if "nc" not in _CACHE:
        _CACHE["nc"] = _build_program()
    return _CACHE["nc"]


def kernel(x, t, y, w, b, trace=False):
    N = x.shape[0]
    assert N == M * R_CORE
    f16 = np.float16
    f32 = np.float32
    x = np.asarray(x, f32)
    t = np.asarray(t, f32).reshape(-1)
    y = np.asarray(y, f32).reshape(-1)

    xtp = np.empty((M, R_G, D), f16)
    ygp = np.empty((M, R_G), f16)
    btp = np.zeros((M, 13, R_T), f16)
    nt_real = R_CORE - R_G
    for i in range(M):
        s0 = i * R_CORE
        xtp[i, :, 0] = t[s0:s0 + R_G]
        xtp[i, :, 1:] = x[s0:s0 + R_G]
        ygp[i] = y[s0:s0 + R_G]
        btp[i, 0, :nt_real] = t[s0 + R_G:s0 + R_CORE]
        btp[i, 1:11, :nt_real] = x[s0 + R_G:s0 + R_CORE].T
        btp[i, 11, :] = 1.0
        btp[i, 12, :nt_real] = y[s0 + R_G:s0 + R_CORE]

    wbi = np.empty(2 * C, f16)
    wbi[0::2] = np.asarray(w, f32)
    wbi[1::2] = np.asarray(b, f32)
    wb_rep = np.ascontiguousarray(np.tile(wbi[None, :], (P, 1)))
    pw_rep = np.ascontiguousarray(
        np.tile(
            np.concatenate([[1024.0], 2.0 ** np.arange(9, -1, -1)]).astype(f16)[None, :],
            (P, 1),
        )
    )
    mk_host = np.zeros((P, 16 * P), f16)
    for k in range(P):
        mk_host[k, (k % 16) * P + k] = 1.0

    l1h = np.zeros((13, 128), f16)
    lyh = np.zeros((13, 128), f16)
    for j in range(32):  # hi combos: (t, x0..x3), MSB first
        pat = [(j >> (4 - i)) & 1 for i in range(5)]
        for i in range(5):
            l1h[i, j] = 2 * pat[i] - 1
        l1h[11, j] = -sum(pat)
    for j in range(64):  # lo combos: (x4..x9), MSB first
        pat = [(j >> (5 - i)) & 1 for i in range(6)]
        for i in range(6):
            l1h[5 + i, 64 + j] = 2 * pat[i] - 1
        l1h[11, 64 + j] = -sum(pat)
    lyh[12, 64:128] = 1.0

    wf = np.asarray(w, f32).reshape(32, 64)
    bf = np.asarray(b, f32).reshape(32, 64)
    t2wh = np.ascontiguousarray(wf.T.astype(f16))   # [64, 32]: [l, h] = w[h*64+l]
    t2bh = np.ascontiguousarray(bf.T.astype(f16))
    on32h = np.ones((32, 1), f16)

    nc = _get_program()
    in_maps = [
        {
            "xt": xtp[i], "y": ygp[i], "wb": wb_rep, "pw": pw_rep,
            "mk": mk_host, "bt": btp[i], "l1": l1h, "ly": lyh,
            "t2w": t2wh, "t2b": t2bh, "on32": on32h,
        }
        for i in range(M)
    ]
    res = run_bass_kernel_spmd(nc, in_maps, core_ids=list(range(M)), trace=trace)
    out = np.empty((N, 1), np.float32)
    for i in range(M):
        s0 = i * R_CORE
        out[s0:s0 + R_G, 0] = res.results[i]["z"]
        out[s0 + R_G:s0 + R_CORE, 0] = res.results[i]["zt2"][:R_CORE - R_G]
    if trace:
        return out, res
    return out


# revision 20
# speedup vs baseline: 1.2565x; 1.2565x over previous
"""Trainium2 Bass kernel for per-combination linear encoder (embedding lookup).

Computes z = y * w[idx] + b[idx] where idx = t*1024 + x @ [512,256,...,1]
for x in {0,1}^[N,10], t in {0,1}^[N,1], over a 2048-entry (w,b) table.

Sharding: data-parallel over the batch axis across 8 NeuronCores; the
tiny (w,b) table is replicated to every core (and every SBUF partition).

v2 (fp16 pipeline): everything 16-bit on the wire.
  - host packs (t, x) into one fp16 [N, 11] array -> single DMA stream and
    one fused 11-wide mult+reduce for the index (values 0/1 and powers of
    two are exact in fp16).
  - GPSIMD ap_gather moves fp16 (w,b) pairs (4 B/idx instead of 8 B),
    halving the Q7 FIFO traffic that dominates the kernel.
  - the 16 accumulating diagonal-mask matmuls that un-wrap the gather
    output run in fp16 (full PE rate instead of fp32's 1/4 rate).
  - DVE FMA z = y*w + b in fp16, output DMA'd as fp16 (host upcasts).

Per-core pipeline (tiles of [128 partitions x B rows]):
  DMA xt/y -> DVE idx (mult 2x + reduce + cast) -> GPSIMD ap_gather ->
  PE un-wrap (32 masked matmuls into PSUM) -> DVE FMA -> DMA out.
The gather is the critical path; sb/gat pools are triple-buffered so all
other engines hide behind it.
"""

import numpy as np

import concourse.bacc as bacc
import concourse.mybir as mybir
from concourse.tile import TileContext
from concourse.bass_utils import run_bass_kernel_spmd

M = 8            # NeuronCores
P = 128          # SBUF partitions
# rows-per-partition schedule; 7 tiles keep the engines pipelined while
# amortizing per-instruction overhead.
B_SCHED = (48, 336, 336, 336, 336, 336, 178, 48)
RPP = sum(B_SCHED)          # rows per partition (1954)
R = P * RPP                 # rows per core (250_112)
D = 11           # bits per row: (t, x0..x9)
C = 2048         # table entries
F16 = mybir.dt.float16
F32 = mybir.dt.float32
I16 = mybir.dt.int16

_CACHE = {}


def _build_program():
    nc = bacc.Bacc("TRN2", target_bir_lowering=False, debug=False, num_devices=M)

    xt = nc.dram_tensor("xt", [R, D], F16, kind="ExternalInput")
    y = nc.dram_tensor("y", [R], F16, kind="ExternalInput")
    wb = nc.dram_tensor("wb", [P, 2 * C], F16, kind="ExternalInput")
    pw = nc.dram_tensor("pw", [P, D], F16, kind="ExternalInput")
    mk = nc.dram_tensor("mk", [P, 16 * P], F16, kind="ExternalInput")
    z = nc.dram_tensor("z", [R], F16, kind="ExternalOutput")

    # row (tile off, partition p, col c) = row (p*RPP + off + c) of the shard
    x3 = xt.ap().rearrange("(pp r) d -> pp (r d)", pp=P)   # [P, RPP*D]
    y2 = y.ap().rearrange("(pp r) -> pp r", pp=P)          # [P, RPP]
    z2 = z.ap().rearrange("(pp r) -> pp r", pp=P)

    with TileContext(nc) as tc:
        with (
            tc.tile_pool(name="const", bufs=1) as cpool,
            tc.tile_pool(name="sb", bufs=3) as pool,
            tc.tile_pool(name="gat", bufs=3) as gpool,
            tc.tile_pool(name="ps", bufs=2, space="PSUM") as ppool,
        ):
            wb_t = cpool.tile([P, 2 * C], F16)
            nc.sync.dma_start(out=wb_t[:], in_=wb[:, :])
            pw_t = cpool.tile([P, D], F16)
            nc.sync.dma_start(out=pw_t[:], in_=pw[:, :])
            mk_t = cpool.tile([P, 16 * P], F16)
            nc.sync.dma_start(out=mk_t[:], in_=mk[:, :])

            off = 0
            for B in B_SCHED:
                xtt = pool.tile([P, B * D], F16, tag="x")
                nc.sync.dma_start(out=xtt[:], in_=x3[:, off * D:(off + B) * D])
                yt = pool.tile([P, B], F16, tag="y")
                nc.sync.dma_start(out=yt[:], in_=y2[:, off:off + B])

                # bits *= powers (in place; powers broadcast along the row dim)
                xv = xtt[:].rearrange("p (b d) -> p b d", d=D)
                nc.vector.tensor_tensor(
                    out=xv, in0=xv,
                    in1=pw_t[:].unsqueeze(1).broadcast_to([P, B, D]),
                    op=mybir.AluOpType.mult,
                )
                # idx = sum_d bit_d * 2^(10-d)  (t is the MSB)
                idxf = pool.tile([P, B], F32, tag="idxf")
                nc.vector.tensor_reduce(
                    out=idxf[:], in_=xv, axis=mybir.AxisListType.X,
                    op=mybir.AluOpType.add,
                )
                idx16 = pool.tile([P, B], I16, tag="idx16")
                nc.vector.tensor_copy(out=idx16[:], in_=idxf[:])

                # gather (w,b) fp16 pairs: og[p, s*16+q, :] = wb[idx16[16k+q, s]]
                og = gpool.tile([P, 16 * B * 2], F16, tag="og")
                nc.gpsimd.ap_gather(
                    out_ap=og[:].rearrange("p (j e) -> p j e", e=2),
                    in_ap=wb_t[:].rearrange("p (c e) -> p c e", e=2),
                    idxs_ap=idx16[:],
                    channels=P, num_elems=C, d=2, num_idxs=16 * B,
                )

                # un-wrap via PE: psum[p, c] = sum_q 1[p%16==q] og[p, (c*16+q)*2+e]
                og3 = og[:].rearrange("p (c s) -> p c s", s=32)
                psw = ppool.tile([P, B], F32, tag="psw")
                psb = ppool.tile([P, B], F32, tag="psb")
                for q in range(16):
                    nc.tensor.matmul(
                        out=psw[:], lhsT=mk_t[:, q * P:(q + 1) * P],
                        rhs=og3[:, :, 2 * q], start=(q == 0), stop=(q == 15),
                    )
                for q in range(16):
                    nc.tensor.matmul(
                        out=psb[:], lhsT=mk_t[:, q * P:(q + 1) * P],
                        rhs=og3[:, :, 2 * q + 1], start=(q == 0), stop=(q == 15),
                    )

                # z = y*w + b
                zt = pool.tile([P, B], F16, tag="z")
                nc.vector.tensor_tensor(
                    out=zt[:], in0=yt[:], in1=psw[:], op=mybir.AluOpType.mult
                )
                nc.vector.tensor_tensor(
                    out=zt[:], in0=zt[:], in1=psb[:], op=mybir.AluOpType.add
                )
                nc.sync.dma_start(out=z2[:, off:off + B], in_=zt[:])
                off += B

    nc.compile()
    return nc


def _get_program():
    if "nc" not in _CACHE:
        _CACHE["nc"] = _build_program()
    return _CACHE["nc"]


def kernel(x, t, y, w, b, trace=False):
    N = x.shape[0]
    npad = M * R - N
    assert npad >= 0
    f16 = np.float16
    # shard rows: core m gets rows [m*R, (m+1)*R); within a core, partition p
    # holds rows [p*RPP, (p+1)*RPP) of its shard, contiguously.
    xtp = np.zeros((M * R, D), f16)
    xtp[:N, 0] = np.asarray(t, np.float32).reshape(-1)
    xtp[:N, 1:] = np.asarray(x, np.float32)
    xtp = xtp.reshape(M, R, D)
    yp = np.concatenate(
        [np.asarray(y, np.float32).reshape(-1), np.zeros(npad, np.float32)]
    ).astype(f16).reshape(M, R)
    wbi = np.empty(2 * C, f16)
    wbi[0::2] = np.asarray(w, np.float32)
    wbi[1::2] = np.asarray(b, np.float32)
    wb_rep = np.ascontiguousarray(np.tile(wbi[None, :], (P, 1)))
    pw_rep = np.ascontiguousarray(
        np.tile(
            np.concatenate(
                [[1024.0], 2.0 ** np.arange(9, -1, -1)]
            ).astype(f16)[None, :],
            (P, 1),
        )
    )
    mk_host = np.zeros((P, 16 * P), f16)
    for k in range(P):
        mk_host[k, (k % 16) * P + k] = 1.0

    nc = _get_program()
    in_maps = [
        {"xt": xtp[i], "y": yp[i], "wb": wb_rep, "pw": pw_rep, "mk": mk_host}
        for i in range(M)
    ]
    res = run_bass_kernel_spmd(nc, in_maps, core_ids=list(range(M)), trace=trace)
    zfull = np.concatenate([res.results[i]["z"] for i in range(M)])[:N]
    out = zfull.reshape(N, 1).astype(np.float32)
    if trace:
        return out, res
    return out
